# revision 9
# baseline (speedup 1.0000x reference)
"""Multi-head attention (B=4, N=2048, C=384, H=12, hd=32) on 8 TRN2 cores.

Sharding: core i handles batch b = i//2 and query half qh = i%2 (1024 query
rows), with full K/V for that batch. No cross-core collectives needed: each
core produces 1024 disjoint output rows.

Per-core kernel (all matmuls bf16, fp32 accumulation in PSUM):
  - inputs arrive pre-transposed from host: x^T [C, T] layouts
  - q^T, k^T computed in [feature, token] layout (head h lives at partition
    offset 32*(h%4) of feature-chunk h//4)
  - v computed in natural [token, feature] layout, augmented with a ones
    column per head (gives the softmax denominator via the same matmul)
  - scores^T = k @ q^T per (head, 128-key chunk) -> PSUM, exp on ScalarE
    (scale folded in), written as bf16 attn^T to SBUF
  - out^T[h] = [v_h | 1]^T @ attn^T accumulated over key chunks -> [33, 512]
  - division by the denominator row via DVE reciprocal + gpsimd partition
    broadcast + DVE multiply, written to attn-out^T [C, token] layout
  - final projection: out[tok, :] = [ao^T | 1]^T @ [W_proj; b_proj]
"""

import os
from contextlib import ExitStack

import numpy as np
import ml_dtypes

import concourse.bass as bass
import concourse.mybir as mybir
import concourse.tile as tile
from concourse import bacc
from concourse.bass_utils import run_bass_kernel_spmd

B, N, C = 4, 2048, 384
H, HD = 12, 32
SCALE = HD ** -0.5
NQ = N // 2          # per-core query rows
NKV = N              # per-core key/value rows
N_CORES = 8
NQB = NQ // 512      # query blocks of 512
NKC = NKV // 128     # key chunks of 128

FP32 = mybir.dt.float32
BF16 = mybir.dt.bfloat16


def _bcast_part(ap, nparts):
    """Partition-broadcast view of a single-partition AP (step-0 partition)."""
    return bass.AP(tensor=ap.tensor, offset=ap.offset, ap=[[0, nparts]] + ap.ap[1:])


def build_nc():
    nc = bacc.Bacc(None)

    x_kvt = nc.dram_tensor("x_kvt", [C, NKV], BF16, kind="ExternalInput")
    x_qt = nc.dram_tensor("x_qt", [C, NQ], BF16, kind="ExternalInput")
    w_qkv = nc.dram_tensor("w_qkv", [C, 3 * C], BF16, kind="ExternalInput")
    w_proj = nc.dram_tensor("w_proj", [C, C], BF16, kind="ExternalInput")
    b_proj = nc.dram_tensor("b_proj", [1, C], BF16, kind="ExternalInput")
    out = nc.dram_tensor("out", [NQ, C], FP32, kind="ExternalOutput")

    with tile.TileContext(nc) as tc, ExitStack() as ctx:
        singles = ctx.enter_context(tc.tile_pool(name="singles", bufs=1))

        # ---- static SBUF tensors -------------------------------------------
        xkv_sb = singles.tile([128, 3 * NKV], BF16)   # x_kv^T, chunk c at cols c*NKV
        xq_sb = singles.tile([128, 3 * NQ], BF16)
        wqkv_sb = singles.tile([128, 3 * 1152], BF16)  # chunk c at cols c*1152
        wp_sb = singles.tile([128, 3 * C], BF16)       # chunk c at cols c*C
        b_sb = singles.tile([1, C], BF16)
        ones_sb = singles.tile([1, 128], BF16)
        kt_sb = singles.tile([128, 3 * NKV], BF16)     # k^T feature-chunk m at cols m*NKV
        qt_sb = singles.tile([128, 3 * NQ], BF16)
        v_sb = singles.tile([128, NKC * (H * 33)], BF16)  # per key-chunk: 12 heads x (32 v + 1 one)
        ao_sb = singles.tile([128, 3 * NQ], BF16)      # attn-out^T, chunk cc at cols cc*NQ

        for c in range(3):
            nc.sync.dma_start(
                out=xkv_sb[:, c * NKV:(c + 1) * NKV], in_=x_kvt[c * 128:(c + 1) * 128, :])
            nc.sync.dma_start(
                out=xq_sb[:, c * NQ:(c + 1) * NQ], in_=x_qt[c * 128:(c + 1) * 128, :])
            nc.sync.dma_start(
                out=wqkv_sb[:, c * 1152:(c + 1) * 1152], in_=w_qkv[c * 128:(c + 1) * 128, :])
            nc.sync.dma_start(
                out=wp_sb[:, c * C:(c + 1) * C], in_=w_proj[c * 128:(c + 1) * 128, :])
        nc.sync.dma_start(out=b_sb[:], in_=b_proj[:])
        nc.vector.memset(ones_sb[:], 1.0)
        # ones columns of v_aug (denominator trick)
        v_ones = v_sb[:].rearrange("p (t h d) -> p t h d", t=NKC, d=33)[:, :, :, 32:33]
        nc.gpsimd.memset(v_ones, 1.0)

        # ---- stage 1: q^T, k^T, v ------------------------------------------
        with tc.tile_pool(name="ps_qkv", bufs=2, space="PSUM") as ps_qkv:
            for dst, src, tspan, woff in ((qt_sb, xq_sb, NQ, 0), (kt_sb, xkv_sb, NKV, C)):
                nblk = tspan // 512
                for m in range(3):
                    for t in range(nblk):
                        pq = ps_qkv.tile([128, 512], FP32, tag="qk")
                        for kc in range(3):
                            nc.tensor.matmul(
                                pq[:],
                                lhsT=wqkv_sb[:, 1152 * kc + woff + 128 * m:
                                             1152 * kc + woff + 128 * (m + 1)],
                                rhs=src[:, tspan * kc + 512 * t:tspan * kc + 512 * (t + 1)],
                                start=(kc == 0), stop=(kc == 2))
                        nc.vector.tensor_copy(
                            out=dst[:, tspan * m + 512 * t:tspan * m + 512 * (t + 1)],
                            in_=pq[:])
            for t in range(NKC):
                pv = ps_qkv.tile([128, C], FP32, tag="v")
                for kc in range(3):
                    nc.tensor.matmul(
                        pv[:],
                        lhsT=xkv_sb[:, NKV * kc + 128 * t:NKV * kc + 128 * (t + 1)],
                        rhs=wqkv_sb[:, 1152 * kc + 768:1152 * kc + 1152],
                        start=(kc == 0), stop=(kc == 2))
                vdst = v_sb[:, t * (H * 33):(t + 1) * (H * 33)].rearrange(
                    "p (h d) -> p h d", d=33)[:, :, 0:32]
                nc.vector.tensor_copy(
                    out=vdst, in_=pv[:].rearrange("p (h d) -> p h d", d=32))

        # ---- stage 2: attention --------------------------------------------
        att_pool = ctx.enter_context(tc.tile_pool(name="att", bufs=2))
        rl_pool = ctx.enter_context(tc.tile_pool(name="rl", bufs=2))
        rlb_pool = ctx.enter_context(tc.tile_pool(name="rlb", bufs=2))
        rld_pool = ctx.enter_context(tc.tile_pool(name="rld", bufs=2, space="DRAM"))
        with tc.tile_pool(name="ps_sc", bufs=1, space="PSUM") as ps_sc, \
             tc.tile_pool(name="ps_av", bufs=2, space="PSUM") as ps_av:
            for qb in range(NQB):
                for hp in range(H // 2):
                    heads = (2 * hp, 2 * hp + 1)
                    att_ts = {}
                    # scores^T + exp, two heads interleaved (different PE row groups)
                    sc_ts = {}
                    for h in heads:
                        att_ts[h] = att_pool.tile([128, NKC * 512], BF16,
                                                  tag=f"att{h % 2}", name=f"att{h % 2}")
                    groups = [(g, min(3, NKC - 3 * g)) for g in range((NKC + 2) // 3)]
                    for g, gsz in groups:
                        for h in heads:
                            m, j = h // 4, h % 4
                            sc = ps_sc.tile([128, 3 * 512], FP32,
                                            tag=f"sc{h % 2}", name=f"sc{h % 2}")
                            sc_ts[h] = sc
                            for ci in range(gsz):
                                c = 3 * g + ci
                                nc.tensor.matmul(
                                    sc[:, 512 * ci:512 * (ci + 1)],
                                    lhsT=kt_sb[32 * j:32 * (j + 1),
                                               NKV * m + 128 * c:NKV * m + 128 * (c + 1)],
                                    rhs=qt_sb[32 * j:32 * (j + 1),
                                              NQ * m + 512 * qb:NQ * m + 512 * (qb + 1)],
                                    start=True, stop=True,
                                    tile_position=(32 * j, 0))
                        for h in heads:
                            nc.scalar.activation(
                                att_ts[h][:, 1536 * g:1536 * g + 512 * gsz],
                                sc_ts[h][:, 0:512 * gsz],
                                mybir.ActivationFunctionType.Exp,
                                scale=SCALE)
                    # attn^T @ [v | 1] and division
                    for h in heads:
                        av = ps_av.tile([128, 512], FP32, tag="av")
                        for c in range(NKC):
                            nc.tensor.matmul(
                                av[0:33, :],
                                lhsT=v_sb[:, c * (H * 33) + 33 * h:c * (H * 33) + 33 * h + 33],
                                rhs=att_ts[h][:, 512 * c:512 * (c + 1)],
                                start=(c == 0), stop=(c == NKC - 1))
                        rl = rl_pool.tile([1, 512], FP32, tag="rl")
                        nc.vector.reciprocal(rl[:], av[32:33, :])
                        # partition-broadcast 1/l via a DRAM bounce (step-0
                        # partition reads are only legal on DRAM APs)
                        rld = rld_pool.tile([1, 512], FP32, tag="rld")
                        nc.sync.dma_start(out=rld[:], in_=rl[:])
                        rlb = rlb_pool.tile([32, 512], FP32, tag="rlb")
                        nc.sync.dma_start(out=rlb[:], in_=_bcast_part(rld[0:1, :], 32))
                        m, j = h // 4, h % 4
                        nc.vector.tensor_mul(
                            ao_sb[32 * j:32 * (j + 1), NQ * m + 512 * qb:NQ * m + 512 * (qb + 1)],
                            av[0:32, :], rlb[:])

        # ---- stage 3: projection -------------------------------------------
        out_pool = ctx.enter_context(tc.tile_pool(name="outp", bufs=2))
        with tc.tile_pool(name="ps_proj", bufs=2, space="PSUM") as ps_proj:
            for t in range(NQ // 128):
                po = ps_proj.tile([128, C], FP32, tag="proj")
                nc.tensor.matmul(po[:], lhsT=ones_sb[0:1, :], rhs=b_sb[0:1, :],
                                 start=True, stop=False)
                for cc in range(3):
                    nc.tensor.matmul(
                        po[:],
                        lhsT=ao_sb[:, NQ * cc + 128 * t:NQ * cc + 128 * (t + 1)],
                        rhs=wp_sb[:, C * cc:C * (cc + 1)],
                        start=False, stop=(cc == 2))
                ot = out_pool.tile([128, C], FP32, tag="out")
                nc.vector.tensor_copy(out=ot[:], in_=po[:])
                nc.sync.dma_start(out=out[128 * t:128 * (t + 1), :], in_=ot[:])

    nc.finalize()
    return nc


_NC_CACHE = None


def _get_nc():
    global _NC_CACHE
    if _NC_CACHE is None:
        _NC_CACHE = build_nc()
    return _NC_CACHE


def _make_in_maps(x, W_qkv, W_proj, b_proj):
    bf = ml_dtypes.bfloat16
    w_qkv_b = np.ascontiguousarray(W_qkv.astype(bf))
    w_proj_b = np.ascontiguousarray(W_proj.astype(bf))
    b_proj_b = np.ascontiguousarray(b_proj.reshape(1, C).astype(bf))
    in_maps = []
    for i in range(N_CORES):
        b, qh = i // 2, i % 2
        xb = np.asarray(x[b], dtype=np.float32)
        in_maps.append({
            "x_kvt": np.ascontiguousarray(xb.T.astype(bf)),
            "x_qt": np.ascontiguousarray(xb[qh * NQ:(qh + 1) * NQ].T.astype(bf)),
            "w_qkv": w_qkv_b,
            "w_proj": w_proj_b,
            "b_proj": b_proj_b,
        })
    return in_maps


def run(x, W_qkv, W_proj, b_proj, trace=False):
    nc = _get_nc()
    in_maps = _make_in_maps(x, W_qkv, W_proj, b_proj)
    res = run_bass_kernel_spmd(nc, in_maps, list(range(N_CORES)), trace=trace)
    out = np.empty((B, N, C), dtype=np.float32)
    for i in range(N_CORES):
        b, qh = i // 2, i % 2
        out[b, qh * NQ:(qh + 1) * NQ] = res.results[i]["out"]
    return out, res


def kernel(x, W_qkv, W_proj, b_proj):
    out, _ = run(x, W_qkv, W_proj, b_proj, trace=False)
    return out
